# revision 12
# baseline (speedup 1.0000x reference)
"""Trainium2 Bass kernel for NearestUpsampling(scale=2) + Conv2d(128->128, 3x3, VALID).

Math: up = nearest_upsample(x, 2); out = conv2d(up, w, VALID) + bias.
Because up[r, c] = x[r//2, c//2], the 3x3 conv over the upsampled image
collapses, per output parity class (p, q) = (row%2, col%2), into a 2x2 conv
over the ORIGINAL image with parity-specific collapsed weights:

    out[2a+p, 2b+q] = sum_{u,v in {0,1}} Wc[p,q,u,v] @ x[a+u, b+v] + bias

where Wc[p,q,u,v] = sum of w[dh,dw] over dh with (p+dh)//2==u and dw with
(q+dw)//2==v.  This cuts MACs 2.25x vs materializing the upsample.

Each of the 4 parity classes is computed as shifted 128x128 matmuls
(cin-contraction) with the 63x63 spatial output as the moving free dim,
accumulating the 4 taps in PSUM.  Parity results are interleaved back into
full output rows by the PSUM->SBUF copy (strided write), with the bias add
fused in, split across the Scalar and Vector engines.  8 cores data-parallel
over the batch dim.
"""

import numpy as np

import concourse.bacc as bacc
import concourse.mybir as mybir
import concourse.tile as tile
from concourse.bass_utils import run_bass_kernel_spmd

N_CORES = 8
B, CIN, H, W = 16, 128, 64, 64
COUT = 128
B_LOC = B // N_CORES  # batch shard per core
HO = WO = 126  # conv output spatial dims (2*64 - 3 + 1)
A = 63  # per-parity output rows/cols
CHUNK = 8  # 'a' rows per PSUM tile (free dim = CHUNK*63 = 504 <= 512)

F32 = mybir.dt.float32
F32R = mybir.dt.float32r


def _round_fp32r(a: np.ndarray) -> np.ndarray:
    """Round fp32 to fp32r (1s+8e+11m, top 20 bits) with round-to-nearest-even.

    The PE's fp32r datapath uses only the top 20 bits of each word; walrus
    requires fp32r-consumed data to be pre-rounded, so we do it on the host.
    """
    u = a.astype(np.float32).view(np.uint32)
    u = u + 0x7FF + ((u >> 12) & 1)
    return (u & 0xFFFFF000).view(np.float32)


def _collapse_weights(weight: np.ndarray) -> np.ndarray:
    """[COUT, CIN, 3, 3] -> [CIN, 16*COUT] collapsed + transposed (lhsT) weights.

    Column block k = ((p*2+q)*2+u)*2+v holds Wc[p,q,u,v] as [cin, cout].
    """
    w = weight.astype(np.float64)
    wc = np.zeros((2, 2, 2, 2, CIN, COUT), np.float64)
    for p in range(2):
        for q in range(2):
            for dh in range(3):
                u = (p + dh) // 2
                for dw in range(3):
                    v = (q + dw) // 2
                    wc[p, q, u, v] += w[:, :, dh, dw].T
    return (
        wc.reshape(16, CIN, COUT)
        .transpose(1, 0, 2)
        .reshape(CIN, 16 * COUT)
        .astype(np.float32)
    )


def build_nc(reps: int = 1):
    """Build and compile the per-core Bass program.

    reps > 1 unrolls the whole kernel body N times (same inputs/outputs) —
    used only for steady-state timing measurements.
    """
    nc = bacc.Bacc("TRN2", target_bir_lowering=False, debug=False, num_devices=N_CORES)

    # x is shipped host-padded to H+1 rows per image (last row zero) so the
    # matmul's 64-wide moving windows can read one element past row H-1.
    xs = nc.dram_tensor("xs", [B_LOC, CIN, (H + 1) * W], F32R, kind="ExternalInput")
    wc = nc.dram_tensor("wc", [CIN, 16 * COUT], F32R, kind="ExternalInput")
    bias = nc.dram_tensor("bias", [COUT, 1], F32, kind="ExternalInput")
    out = nc.dram_tensor("out", [B_LOC, COUT, HO, WO], F32, kind="ExternalOutput")

    with tile.TileContext(nc) as tc:
        with (
            tc.tile_pool(name="wpool", bufs=1) as wpool,
            tc.tile_pool(name="xpool", bufs=2) as xpool,
            tc.tile_pool(name="opool", bufs=3) as opool,
            tc.tile_pool(name="pspool", bufs=8, space="PSUM") as pspool,
        ):
            w_t = wpool.tile([128, 16 * COUT], F32R)
            nc.sync.dma_start(w_t[:], wc.ap())
            b_t = wpool.tile([128, 1], F32)
            nc.sync.dma_start(b_t[:], bias.ap())

            for b in [b for _ in range(reps) for b in range(B_LOC)]:
                # One extra (zeroed) row so 64-wide windows at col offset 1 can
                # read one element past the last row: the fp32r moving operand
                # needs an even inner count, so we read 64 cols and drop col 63.
                x_t = xpool.tile([128, (H + 1) * W], F32R)
                nc.sync.dma_start(x_t[:], xs.ap()[b])

                for a0 in range(0, A, CHUNK):
                    na = min(CHUNK, A - a0)
                    o_t = opool.tile([128, 2 * na * WO], F32, tag="o")
                    o_v = o_t[:].rearrange(
                        "c (a r j w2) -> c a r j w2", a=na, r=2, j=A, w2=2
                    )
                    for pi, (p, q) in enumerate(((0, 0), (0, 1), (1, 0), (1, 1))):
                        ps = pspool.tile([128, na * W], F32, tag="ps")
                        for ti, (u, v) in enumerate(((0, 0), (0, 1), (1, 0), (1, 1))):
                            k = ((p * 2 + q) * 2 + u) * 2 + v
                            lhsT = w_t[:, k * COUT : (k + 1) * COUT]
                            off = (a0 + u) * W + v
                            rhs = x_t[:, off : off + na * W].rearrange(
                                "c (a j) -> c a j", a=na
                            )
                            nc.tensor.matmul(
                                ps[:], lhsT, rhs, start=(ti == 0), stop=(ti == 3)
                            )
                        src = ps[:].rearrange("c (a j) -> c a j", a=na)[:, :, 0:A]
                        dst = o_v[:, :, p, :, q]
                        if pi % 2 == 0:
                            nc.scalar.activation(
                                dst,
                                src,
                                mybir.ActivationFunctionType.Identity,
                                bias=b_t[:, 0:1],
                            )
                        else:
                            nc.vector.tensor_scalar_add(dst, src, b_t[:, 0:1])
                    nc.sync.dma_start(
                        out.ap()[b].rearrange("c h w -> c (h w)")[
                            :, 2 * a0 * WO : (2 * a0 + 2 * na) * WO
                        ],
                        o_t[:],
                    )

    nc.compile()
    return nc


_NC = None


def _get_nc():
    global _NC
    if _NC is None:
        _NC = build_nc()
    return _NC


def make_in_maps(x, weight, bias):
    wc = _round_fp32r(_collapse_weights(np.asarray(weight)))
    b = np.ascontiguousarray(np.asarray(bias, dtype=np.float32).reshape(COUT, 1))
    xr = _round_fp32r(np.asarray(x, dtype=np.float32))
    xp = np.zeros((B, CIN, (H + 1) * W), np.float32)
    xp[:, :, : H * W] = xr.reshape(B, CIN, H * W)
    return [
        {"xs": xp[i * B_LOC : (i + 1) * B_LOC], "wc": wc, "bias": b}
        for i in range(N_CORES)
    ]


def kernel(x, weight, bias):
    nc = _get_nc()
    in_maps = make_in_maps(x, weight, bias)
    res = run_bass_kernel_spmd(nc, in_maps, core_ids=list(range(N_CORES)))
    return np.concatenate([res.results[i]["out"] for i in range(N_CORES)], axis=0)


# revision 17
# speedup vs baseline: 1.9405x; 1.9405x over previous
"""Trainium2 Bass kernel for NearestUpsampling(scale=2) + Conv2d(128->128, 3x3, VALID).

Math: up = nearest_upsample(x, 2); out = conv2d(up, w, VALID) + bias.
Because up[r, c] = x[r//2, c//2], the 3x3 conv over the upsampled image
collapses, per output parity class (p, q) = (row%2, col%2), into a 2x2 conv
over the ORIGINAL image with parity-specific collapsed weights:

    out[2a+p, 2b+q] = sum_{u,v in {0,1}} Wc[p,q,u,v] @ x[a+u, b+v] + bias

where Wc[p,q,u,v] = sum of w[dh,dw] over dh with (p+dh)//2==u and dw with
(q+dw)//2==v.  This cuts MACs 2.25x vs materializing the upsample.

Each of the 4 parity classes is computed as shifted 128x128 matmuls
(cin-contraction) with the 63x63 spatial output as the moving free dim,
accumulating the 4 taps in PSUM.  Parity results are interleaved back into
full output rows by the PSUM->SBUF copy (strided write), with the bias add
fused in, split across the Scalar and Vector engines.  8 cores data-parallel
over the batch dim.
"""

import numpy as np

import concourse.bacc as bacc
import concourse.mybir as mybir
import concourse.tile as tile
from concourse.bass_utils import run_bass_kernel_spmd

N_CORES = 8
B, CIN, H, W = 16, 128, 64, 64
COUT = 128
B_LOC = B // N_CORES  # batch shard per core
HO = WO = 126  # conv output spatial dims (2*64 - 3 + 1)
A = 63  # per-parity output rows/cols
CHUNK = 8  # 'a' rows per PSUM tile (free dim = CHUNK*63 = 504 <= 512)

F32 = mybir.dt.float32
F32R = mybir.dt.float32r


def _round_fp32r(a: np.ndarray) -> np.ndarray:
    """Round fp32 to fp32r (1s+8e+11m, top 20 bits) with round-to-nearest-even.

    The PE's fp32r datapath uses only the top 20 bits of each word; walrus
    requires fp32r-consumed data to be pre-rounded, so we do it on the host.
    """
    u = a.astype(np.float32).view(np.uint32)
    u = u + 0x7FF + ((u >> 12) & 1)
    return (u & 0xFFFFF000).view(np.float32)


def _collapse_weights(weight: np.ndarray) -> np.ndarray:
    """[COUT, CIN, 3, 3] -> [CIN, 16*COUT] collapsed + transposed (lhsT) weights.

    Column block k = ((p*2+q)*2+u)*2+v holds Wc[p,q,u,v] as [cin, cout].
    """
    w = weight.astype(np.float64)
    wc = np.zeros((2, 2, 2, 2, CIN, COUT), np.float64)
    for p in range(2):
        for q in range(2):
            for dh in range(3):
                u = (p + dh) // 2
                for dw in range(3):
                    v = (q + dw) // 2
                    wc[p, q, u, v] += w[:, :, dh, dw].T
    return (
        wc.reshape(16, CIN, COUT)
        .transpose(1, 0, 2)
        .reshape(CIN, 16 * COUT)
        .astype(np.float32)
    )


def build_nc(reps: int = 1, loop_n: int = 1):
    """Build and compile the per-core Bass program.

    reps > 1 unrolls the kernel body N times; loop_n > 1 wraps the body in a
    hardware For_i loop.  Both are used only for timing measurements.
    """
    from contextlib import ExitStack

    nc = bacc.Bacc("TRN2", target_bir_lowering=False, debug=False, num_devices=N_CORES)

    # x is shipped host-padded to H+1 rows per image (last row zero) so the
    # matmul's 64-wide moving windows can read one element past row H-1.
    xs = nc.dram_tensor("xs", [B_LOC, CIN, (H + 1) * W], F32R, kind="ExternalInput")
    wc = nc.dram_tensor("wc", [CIN, 16 * COUT], F32R, kind="ExternalInput")
    bias = nc.dram_tensor("bias", [COUT, 1], F32, kind="ExternalInput")
    out = nc.dram_tensor("out", [B_LOC, COUT, HO, WO], F32, kind="ExternalOutput")

    # x is split per image into a head tile (rows 0..17) and a tail tile
    # (rows 16..65) so the first chunks' matmuls can start before the whole
    # image has landed in SBUF.
    LO_ROWS = 18  # rows 0..17 cover chunks a0=0 and a0=8
    HI_ROW0 = 16  # tail tile starts at row 16

    with tile.TileContext(nc) as tc:
        with (
            tc.tile_pool(name="wpool", bufs=1) as wpool,
            tc.tile_pool(name="xpool", bufs=2) as xpool,
            tc.tile_pool(name="opool", bufs=3) as opool,
            tc.tile_pool(name="pspool", bufs=7, space="PSUM") as pspool,
            tc.tile_pool(name="warmpool", bufs=1, space="PSUM") as warmpool,
        ):
            # PE warmup: fp32 matmuls (4 cyc/row) on a zeroed scratch tile into
            # a dead PSUM bank, issued while the weight/x DMAs are in flight,
            # so the HAM clock gate reaches full rate before the first real
            # matmul.  ~7 x 1.7us covers the DMA latency + HAM window.
            warm_src = wpool.tile([128, 512], F32, name="warm_src")
            nc.vector.memset(warm_src[:], 0.0)
            warm_ps = warmpool.tile([128, 512], F32, name="warm_ps")
            for _ in range(7):
                nc.tensor.matmul(
                    warm_ps[:],
                    warm_src[:, 0:128],
                    warm_src[:],
                    start=True,
                    stop=True,
                    skip_group_check=True,
                )

            w_t = wpool.tile([128, 16 * COUT], F32R)
            nc.sync.dma_start(w_t[:], wc.ap())
            b_t = wpool.tile([128, 1], F32)
            nc.sync.dma_start(b_t[:], bias.ap())

            stack = ExitStack()
            if loop_n > 1:
                stack.enter_context(tc.For_i(0, loop_n, 1))

            for b in [b for _ in range(reps) for b in range(B_LOC)]:
                # One extra (zeroed) row so 64-wide windows at col offset 1 can
                # read one element past the last row: the fp32r moving operand
                # needs an even inner count, so we read 64 cols and drop col 63.
                x_lo = xpool.tile([128, LO_ROWS * W], F32R, tag="xlo")
                nc.sync.dma_start(x_lo[:], xs.ap()[b][:, 0 : LO_ROWS * W])
                x_hi = xpool.tile([128, (H + 1 - HI_ROW0) * W], F32R, tag="xhi")
                nc.sync.dma_start(x_hi[:], xs.ap()[b][:, HI_ROW0 * W :])

                for a0 in range(0, A, CHUNK):
                    na = min(CHUNK, A - a0)
                    o_t = opool.tile([128, 2 * na * WO], F32, tag="o")
                    o_v = o_t[:].rearrange(
                        "c (a r j w2) -> c a r j w2", a=na, r=2, j=A, w2=2
                    )
                    for pi, (p, q) in enumerate(((0, 0), (0, 1), (1, 0), (1, 1))):
                        ps = pspool.tile([128, na * W], F32, tag="ps")
                        for ti, (u, v) in enumerate(((0, 0), (0, 1), (1, 0), (1, 1))):
                            k = ((p * 2 + q) * 2 + u) * 2 + v
                            lhsT = w_t[:, k * COUT : (k + 1) * COUT]
                            if a0 + u + na <= LO_ROWS - 1:
                                x_src, off = x_lo, (a0 + u) * W + v
                            else:
                                x_src, off = x_hi, (a0 + u - HI_ROW0) * W + v
                            rhs = x_src[:, off : off + na * W].rearrange(
                                "c (a j) -> c a j", a=na
                            )
                            nc.tensor.matmul(
                                ps[:], lhsT, rhs, start=(ti == 0), stop=(ti == 3)
                            )
                        src = ps[:].rearrange("c (a j) -> c a j", a=na)[:, :, 0:A]
                        dst = o_v[:, :, p, :, q]
                        if pi % 2 == 0:
                            nc.scalar.activation(
                                dst,
                                src,
                                mybir.ActivationFunctionType.Identity,
                                bias=b_t[:, 0:1],
                            )
                        else:
                            nc.vector.tensor_scalar_add(dst, src, b_t[:, 0:1])
                    nc.sync.dma_start(
                        out.ap()[b].rearrange("c h w -> c (h w)")[
                            :, 2 * a0 * WO : (2 * a0 + 2 * na) * WO
                        ],
                        o_t[:],
                    )
            stack.close()

    nc.compile()
    return nc


_NC = None


def _get_nc():
    global _NC
    if _NC is None:
        _NC = build_nc()
    return _NC


def make_in_maps(x, weight, bias):
    wc = _round_fp32r(_collapse_weights(np.asarray(weight)))
    b = np.ascontiguousarray(np.asarray(bias, dtype=np.float32).reshape(COUT, 1))
    xr = _round_fp32r(np.asarray(x, dtype=np.float32))
    xp = np.zeros((B, CIN, (H + 1) * W), np.float32)
    xp[:, :, : H * W] = xr.reshape(B, CIN, H * W)
    return [
        {"xs": xp[i * B_LOC : (i + 1) * B_LOC], "wc": wc, "bias": b}
        for i in range(N_CORES)
    ]


def kernel(x, weight, bias):
    nc = _get_nc()
    in_maps = make_in_maps(x, weight, bias)
    res = run_bass_kernel_spmd(nc, in_maps, core_ids=list(range(N_CORES)))
    return np.concatenate([res.results[i]["out"] for i in range(N_CORES)], axis=0)


# revision 19
# speedup vs baseline: 1.9573x; 1.0087x over previous
"""Trainium2 Bass kernel for NearestUpsampling(scale=2) + Conv2d(128->128, 3x3, VALID).

Math: up = nearest_upsample(x, 2); out = conv2d(up, w, VALID) + bias.
Because up[r, c] = x[r//2, c//2], the 3x3 conv over the upsampled image
collapses, per output parity class (p, q) = (row%2, col%2), into a 2x2 conv
over the ORIGINAL image with parity-specific collapsed weights:

    out[2a+p, 2b+q] = sum_{u,v in {0,1}} Wc[p,q,u,v] @ x[a+u, b+v] + bias

where Wc[p,q,u,v] = sum of w[dh,dw] over dh with (p+dh)//2==u and dw with
(q+dw)//2==v.  This cuts MACs 2.25x vs materializing the upsample.

Each of the 4 parity classes is computed as shifted 128x128 matmuls
(cin-contraction) with the 63x63 spatial output as the moving free dim,
accumulating the 4 taps in PSUM.  Parity results are interleaved back into
full output rows by the PSUM->SBUF copy (strided write), with the bias add
fused in, split across the Scalar and Vector engines.  8 cores data-parallel
over the batch dim.
"""

import numpy as np

import concourse.bacc as bacc
import concourse.mybir as mybir
import concourse.tile as tile
from concourse.bass_utils import run_bass_kernel_spmd

N_CORES = 8
B, CIN, H, W = 16, 128, 64, 64
COUT = 128
B_LOC = B // N_CORES  # batch shard per core
HO = WO = 126  # conv output spatial dims (2*64 - 3 + 1)
A = 63  # per-parity output rows/cols
CHUNK = 8  # 'a' rows per PSUM tile (free dim = CHUNK*63 = 504 <= 512)

F32 = mybir.dt.float32
F32R = mybir.dt.float32r


def _round_fp32r(a: np.ndarray) -> np.ndarray:
    """Round fp32 to fp32r (1s+8e+11m, top 20 bits) with round-to-nearest-even.

    The PE's fp32r datapath uses only the top 20 bits of each word; walrus
    requires fp32r-consumed data to be pre-rounded, so we do it on the host.
    """
    u = a.astype(np.float32).view(np.uint32)
    u = u + 0x7FF + ((u >> 12) & 1)
    return (u & 0xFFFFF000).view(np.float32)


def _collapse_weights(weight: np.ndarray) -> np.ndarray:
    """[COUT, CIN, 3, 3] -> [CIN, 16*COUT] collapsed + transposed (lhsT) weights.

    Column block k = ((p*2+q)*2+u)*2+v holds Wc[p,q,u,v] as [cin, cout].
    """
    w = weight.astype(np.float64)
    wc = np.zeros((2, 2, 2, 2, CIN, COUT), np.float64)
    for p in range(2):
        for q in range(2):
            for dh in range(3):
                u = (p + dh) // 2
                for dw in range(3):
                    v = (q + dw) // 2
                    wc[p, q, u, v] += w[:, :, dh, dw].T
    return (
        wc.reshape(16, CIN, COUT)
        .transpose(1, 0, 2)
        .reshape(CIN, 16 * COUT)
        .astype(np.float32)
    )


def build_nc(reps: int = 1, loop_n: int = 1):
    """Build and compile the per-core Bass program.

    reps > 1 unrolls the kernel body N times; loop_n > 1 wraps the body in a
    hardware For_i loop.  Both are used only for timing measurements.
    """
    from contextlib import ExitStack

    nc = bacc.Bacc("TRN2", target_bir_lowering=False, debug=False, num_devices=N_CORES)

    # x is shipped host-padded to H+1 rows per image (last row zero) so the
    # matmul's 64-wide moving windows can read one element past row H-1.
    xs = nc.dram_tensor("xs", [B_LOC, CIN, (H + 1) * W], F32R, kind="ExternalInput")
    wc = nc.dram_tensor("wc", [CIN, 16 * COUT], F32R, kind="ExternalInput")
    bias = nc.dram_tensor("bias", [COUT, 1], F32, kind="ExternalInput")
    out = nc.dram_tensor("out", [B_LOC, COUT, HO, WO], F32, kind="ExternalOutput")

    # x is split per image into a head tile (rows 0..17) and a tail tile
    # (rows 16..65) so the first chunks' matmuls can start before the whole
    # image has landed in SBUF.
    LO_ROWS = 18  # rows 0..17 cover chunks a0=0 and a0=8
    HI_ROW0 = 16  # tail tile starts at row 16

    with tile.TileContext(nc) as tc:
        with (
            tc.tile_pool(name="wpool", bufs=1) as wpool,
            tc.tile_pool(name="xpool", bufs=2) as xpool,
            tc.tile_pool(name="opool", bufs=3) as opool,
            tc.tile_pool(name="pspool", bufs=8, space="PSUM") as pspool,
        ):
            # PE warmup: fp32 matmuls (4 cyc/row) on a zeroed scratch tile into
            # a dead PSUM bank, issued while the weight/x DMAs are in flight,
            # so the HAM clock gate reaches full rate before the first real
            # matmul.  ~7 x 1.7us covers the DMA latency + HAM window.
            warm_src = wpool.tile([128, 512], F32, name="warm_src")
            nc.vector.memset(warm_src[:], 0.0)
            # The warm PSUM tile shares the "ps" slot rotation; its slot is
            # recycled once the warmup matmuls retire.
            warm_ps = pspool.tile([128, 512], F32, tag="ps", name="warm_ps")
            for _ in range(7):
                nc.tensor.matmul(
                    warm_ps[:],
                    warm_src[:, 0:128],
                    warm_src[:],
                    start=True,
                    stop=True,
                    skip_group_check=True,
                )

            w_t = wpool.tile([128, 16 * COUT], F32R)
            nc.sync.dma_start(w_t[:], wc.ap())
            b_t = wpool.tile([128, 1], F32)
            nc.sync.dma_start(b_t[:], bias.ap())

            stack = ExitStack()
            if loop_n > 1:
                stack.enter_context(tc.For_i(0, loop_n, 1))

            for b in [b for _ in range(reps) for b in range(B_LOC)]:
                # One extra (zeroed) row so 64-wide windows at col offset 1 can
                # read one element past the last row: the fp32r moving operand
                # needs an even inner count, so we read 64 cols and drop col 63.
                x_lo = xpool.tile([128, LO_ROWS * W], F32R, tag="xlo")
                nc.sync.dma_start(x_lo[:], xs.ap()[b][:, 0 : LO_ROWS * W])
                x_hi = xpool.tile([128, (H + 1 - HI_ROW0) * W], F32R, tag="xhi")
                nc.sync.dma_start(x_hi[:], xs.ap()[b][:, HI_ROW0 * W :])

                for a0 in range(0, A, CHUNK):
                    na = min(CHUNK, A - a0)
                    o_t = opool.tile([128, 2 * na * WO], F32, tag="o")
                    o_v = o_t[:].rearrange(
                        "c (a r j w2) -> c a r j w2", a=na, r=2, j=A, w2=2
                    )
                    for pi, (p, q) in enumerate(((0, 0), (0, 1), (1, 0), (1, 1))):
                        ps = pspool.tile([128, na * W], F32, tag="ps")
                        for ti, (u, v) in enumerate(((0, 0), (0, 1), (1, 0), (1, 1))):
                            k = ((p * 2 + q) * 2 + u) * 2 + v
                            lhsT = w_t[:, k * COUT : (k + 1) * COUT]
                            if a0 + u + na <= LO_ROWS - 1:
                                x_src, off = x_lo, (a0 + u) * W + v
                            else:
                                x_src, off = x_hi, (a0 + u - HI_ROW0) * W + v
                            rhs = x_src[:, off : off + na * W].rearrange(
                                "c (a j) -> c a j", a=na
                            )
                            nc.tensor.matmul(
                                ps[:], lhsT, rhs, start=(ti == 0), stop=(ti == 3)
                            )
                        src = ps[:].rearrange("c (a j) -> c a j", a=na)[:, :, 0:A]
                        dst = o_v[:, :, p, :, q]
                        if pi % 2 == 0:
                            nc.scalar.activation(
                                dst,
                                src,
                                mybir.ActivationFunctionType.Identity,
                                bias=b_t[:, 0:1],
                            )
                        else:
                            nc.vector.tensor_scalar_add(dst, src, b_t[:, 0:1])
                    nc.sync.dma_start(
                        out.ap()[b].rearrange("c h w -> c (h w)")[
                            :, 2 * a0 * WO : (2 * a0 + 2 * na) * WO
                        ],
                        o_t[:],
                    )
            stack.close()

    nc.compile()
    return nc


_NC = None


def _get_nc():
    global _NC
    if _NC is None:
        _NC = build_nc()
    return _NC


def make_in_maps(x, weight, bias):
    wc = _round_fp32r(_collapse_weights(np.asarray(weight)))
    b = np.ascontiguousarray(np.asarray(bias, dtype=np.float32).reshape(COUT, 1))
    xr = _round_fp32r(np.asarray(x, dtype=np.float32))
    xp = np.zeros((B, CIN, (H + 1) * W), np.float32)
    xp[:, :, : H * W] = xr.reshape(B, CIN, H * W)
    return [
        {"xs": xp[i * B_LOC : (i + 1) * B_LOC], "wc": wc, "bias": b}
        for i in range(N_CORES)
    ]


def kernel(x, weight, bias):
    nc = _get_nc()
    in_maps = make_in_maps(x, weight, bias)
    res = run_bass_kernel_spmd(nc, in_maps, core_ids=list(range(N_CORES)))
    return np.concatenate([res.results[i]["out"] for i in range(N_CORES)], axis=0)


# revision 20
# speedup vs baseline: 1.9573x; 1.0000x over previous
"""Trainium2 Bass kernel for NearestUpsampling(scale=2) + Conv2d(128->128, 3x3, VALID).

Math: up = nearest_upsample(x, 2); out = conv2d(up, w, VALID) + bias.
Because up[r, c] = x[r//2, c//2], the 3x3 conv over the upsampled image
collapses, per output parity class (p, q) = (row%2, col%2), into a 2x2 conv
over the ORIGINAL image with parity-specific collapsed weights:

    out[2a+p, 2b+q] = sum_{u,v in {0,1}} Wc[p,q,u,v] @ x[a+u, b+v] + bias

where Wc[p,q,u,v] = sum of w[dh,dw] over dh with (p+dh)//2==u and dw with
(q+dw)//2==v.  This cuts MACs 2.25x vs materializing the upsample.

Each of the 4 parity classes is computed as shifted 128x128 matmuls
(cin-contraction) with the 63x63 spatial output as the moving free dim,
accumulating the 4 taps in PSUM.  Parity results are interleaved back into
full output rows by the PSUM->SBUF copy (strided write), with the bias add
fused in, split across the Scalar and Vector engines.  8 cores data-parallel
over the batch dim.
"""

import numpy as np

import concourse.bacc as bacc
import concourse.mybir as mybir
import concourse.tile as tile
from concourse.bass_utils import run_bass_kernel_spmd

N_CORES = 8
B, CIN, H, W = 16, 128, 64, 64
COUT = 128
B_LOC = B // N_CORES  # batch shard per core
HO = WO = 126  # conv output spatial dims (2*64 - 3 + 1)
A = 63  # per-parity output rows/cols
CHUNK = 8  # 'a' rows per PSUM tile (free dim = CHUNK*63 = 504 <= 512)

F32 = mybir.dt.float32
F32R = mybir.dt.float32r


def _round_fp32r(a: np.ndarray) -> np.ndarray:
    """Round fp32 to fp32r (1s+8e+11m, top 20 bits) with round-to-nearest-even.

    The PE's fp32r datapath uses only the top 20 bits of each word; walrus
    requires fp32r-consumed data to be pre-rounded, so we do it on the host.
    """
    u = a.astype(np.float32).view(np.uint32)
    u = u + 0x7FF + ((u >> 12) & 1)
    return (u & 0xFFFFF000).view(np.float32)


def _collapse_weights(weight: np.ndarray) -> np.ndarray:
    """[COUT, CIN, 3, 3] -> [CIN, 16*COUT] collapsed + transposed (lhsT) weights.

    Column block k = ((p*2+q)*2+u)*2+v holds Wc[p,q,u,v] as [cin, cout].
    """
    w = weight.astype(np.float64)
    wc = np.zeros((2, 2, 2, 2, CIN, COUT), np.float64)
    for p in range(2):
        for q in range(2):
            for dh in range(3):
                u = (p + dh) // 2
                for dw in range(3):
                    v = (q + dw) // 2
                    wc[p, q, u, v] += w[:, :, dh, dw].T
    return (
        wc.reshape(16, CIN, COUT)
        .transpose(1, 0, 2)
        .reshape(CIN, 16 * COUT)
        .astype(np.float32)
    )


def build_nc(reps: int = 1, loop_n: int = 1):
    """Build and compile the per-core Bass program.

    reps > 1 unrolls the kernel body N times; loop_n > 1 wraps the body in a
    hardware For_i loop.  Both are used only for timing measurements.
    """
    from contextlib import ExitStack

    nc = bacc.Bacc("TRN2", target_bir_lowering=False, debug=False, num_devices=N_CORES)

    # x is shipped host-padded to H+1 rows per image (last row zero) so the
    # matmul's 64-wide moving windows can read one element past row H-1.
    xs = nc.dram_tensor("xs", [B_LOC, CIN, (H + 1) * W], F32R, kind="ExternalInput")
    wc = nc.dram_tensor("wc", [CIN, 16 * COUT], F32R, kind="ExternalInput")
    bias = nc.dram_tensor("bias", [COUT, 1], F32, kind="ExternalInput")
    out = nc.dram_tensor("out", [B_LOC, COUT, HO, WO], F32, kind="ExternalOutput")

    # x is split per image into a head tile (rows 0..17) and a tail tile
    # (rows 16..65) so the first chunks' matmuls can start before the whole
    # image has landed in SBUF.
    LO_ROWS = 18  # rows 0..17 cover chunks a0=0 and a0=8
    HI_ROW0 = 16  # tail tile starts at row 16

    with tile.TileContext(nc) as tc:
        with (
            tc.tile_pool(name="wpool", bufs=1) as wpool,
            tc.tile_pool(name="xpool", bufs=2) as xpool,
            tc.tile_pool(name="opool", bufs=3) as opool,
            tc.tile_pool(name="pspool", bufs=8, space="PSUM") as pspool,
        ):
            # PE warmup: fp32 matmuls (4 cyc/row) on a zeroed scratch tile into
            # a dead PSUM bank, issued while the weight/x DMAs are in flight,
            # so the HAM clock gate reaches full rate before the first real
            # matmul.  ~7 x 1.7us covers the DMA latency + HAM window.
            warm_src = wpool.tile([128, 512], F32, name="warm_src")
            nc.vector.memset(warm_src[:], 0.0)
            # The warm PSUM tile shares the "ps" slot rotation; its slot is
            # recycled once the warmup matmuls retire.
            warm_ps = pspool.tile([128, 512], F32, tag="ps", name="warm_ps")
            for _ in range(7):
                nc.tensor.matmul(
                    warm_ps[:],
                    warm_src[:, 0:128],
                    warm_src[:],
                    start=True,
                    stop=True,
                    skip_group_check=True,
                )

            w_t = wpool.tile([128, 16 * COUT], F32R)
            nc.sync.dma_start(w_t[:], wc.ap())
            b_t = wpool.tile([128, 1], F32)
            nc.sync.dma_start(b_t[:], bias.ap())

            stack = ExitStack()
            if loop_n > 1:
                stack.enter_context(
                    tc.For_i(
                        0, loop_n, 1, hint_engines=(mybir.EngineType.PE,)
                    )
                )

            for b in [b for _ in range(reps) for b in range(B_LOC)]:
                # One extra (zeroed) row so 64-wide windows at col offset 1 can
                # read one element past the last row: the fp32r moving operand
                # needs an even inner count, so we read 64 cols and drop col 63.
                x_lo = xpool.tile([128, LO_ROWS * W], F32R, tag="xlo")
                nc.sync.dma_start(x_lo[:], xs.ap()[b][:, 0 : LO_ROWS * W])
                x_hi = xpool.tile([128, (H + 1 - HI_ROW0) * W], F32R, tag="xhi")
                nc.sync.dma_start(x_hi[:], xs.ap()[b][:, HI_ROW0 * W :])

                for a0 in range(0, A, CHUNK):
                    na = min(CHUNK, A - a0)
                    o_t = opool.tile([128, 2 * na * WO], F32, tag="o")
                    o_v = o_t[:].rearrange(
                        "c (a r j w2) -> c a r j w2", a=na, r=2, j=A, w2=2
                    )
                    for pi, (p, q) in enumerate(((0, 0), (0, 1), (1, 0), (1, 1))):
                        ps = pspool.tile([128, na * W], F32, tag="ps")
                        for ti, (u, v) in enumerate(((0, 0), (0, 1), (1, 0), (1, 1))):
                            k = ((p * 2 + q) * 2 + u) * 2 + v
                            lhsT = w_t[:, k * COUT : (k + 1) * COUT]
                            if a0 + u + na <= LO_ROWS - 1:
                                x_src, off = x_lo, (a0 + u) * W + v
                            else:
                                x_src, off = x_hi, (a0 + u - HI_ROW0) * W + v
                            rhs = x_src[:, off : off + na * W].rearrange(
                                "c (a j) -> c a j", a=na
                            )
                            nc.tensor.matmul(
                                ps[:], lhsT, rhs, start=(ti == 0), stop=(ti == 3)
                            )
                        src = ps[:].rearrange("c (a j) -> c a j", a=na)[:, :, 0:A]
                        dst = o_v[:, :, p, :, q]
                        if pi % 2 == 0:
                            nc.scalar.activation(
                                dst,
                                src,
                                mybir.ActivationFunctionType.Identity,
                                bias=b_t[:, 0:1],
                            )
                        else:
                            nc.vector.tensor_scalar_add(dst, src, b_t[:, 0:1])
                    nc.sync.dma_start(
                        out.ap()[b].rearrange("c h w -> c (h w)")[
                            :, 2 * a0 * WO : (2 * a0 + 2 * na) * WO
                        ],
                        o_t[:],
                    )
            stack.close()

    nc.compile()
    return nc


_NC = None


def _get_nc():
    global _NC
    if _NC is None:
        _NC = build_nc()
    return _NC


def make_in_maps(x, weight, bias):
    wc = _round_fp32r(_collapse_weights(np.asarray(weight)))
    b = np.ascontiguousarray(np.asarray(bias, dtype=np.float32).reshape(COUT, 1))
    xr = _round_fp32r(np.asarray(x, dtype=np.float32))
    xp = np.zeros((B, CIN, (H + 1) * W), np.float32)
    xp[:, :, : H * W] = xr.reshape(B, CIN, H * W)
    return [
        {"xs": xp[i * B_LOC : (i + 1) * B_LOC], "wc": wc, "bias": b}
        for i in range(N_CORES)
    ]


def kernel(x, weight, bias):
    nc = _get_nc()
    in_maps = make_in_maps(x, weight, bias)
    res = run_bass_kernel_spmd(nc, in_maps, core_ids=list(range(N_CORES)))
    return np.concatenate([res.results[i]["out"] for i in range(N_CORES)], axis=0)
